# revision 15
# baseline (speedup 1.0000x reference)
"""DeepFM embedding lookup kernel for 8 TRN2 NeuronCores.

Problem shapes (hardcoded, self-contained):
  Xi (512, 32, 39, 1) int32, Xv (512, 32, 39) f32
  w1_lin/w2_lin (16, 1), b1_lin/b2_lin (16,)
  T1/T2 (38, 100000, 16) f32
Returns (fm_first (512, 32, 624) f32, fm_second (16384, 39, 16) f32)

Sharding: the 38 categorical fields are sharded across the 8 cores, 5
field-slots per core (ranges overlap by one field on cores 6/7; overlap
slots are deactivated with xv=0 and ignored at unshard). T1/T2 rows for
the same (field, vocab) are interleaved host-side into one (.., 32) table
so one 128B gather descriptor fetches both orders' embedding rows.
Each core gathers 16384 rows x 5 fields with one flat row index per
lookup via indirect DMA, scales by Xv on DVE, and writes back. The tiny
continuous-field linear is row-sharded (2048 rows/core).
"""

import numpy as np
import sys

for p in ("/root/.axon_site", "/root/.axon_site/_ro/trn_rl_repo", "/root/.axon_site/_ro/pypackages", "/opt/trn_rl_repo"):
    if p not in sys.path:
        sys.path.append(p)

import concourse.bass as bass
import concourse.bacc as bacc
import concourse.mybir as mybir
from concourse.tile import TileContext
from concourse.bass_utils import run_bass_kernel_spmd

F32 = mybir.dt.float32
I32 = mybir.dt.int32

VOCAB = 100000
E = 16
F_CAT = 38
B, L, C = 512, 32, 39
BL = B * L                    # 16384
P = 128                       # SBUF partitions
NF = 5                        # field slots per core
SLOTS = BL * NF // P          # 640 lookup slots per partition
RPP = BL // P                 # 128 rows per partition
EE = 2 * E                    # 32 f32 per gathered element (T1 row | T2 row)
CHUNKS = [32, 64, 96, 128, 128, 96, 64, 32]   # slots per chunk, sum = SLOTS
assert sum(CHUNKS) == SLOTS
FUSE_CCE = False              # HW: "DMACopy does not support mult with Copy mode"
RC = BL // P                  # rows per partition
CONT_R = BL // 8              # 2048 rows of continuous field per core
CONT_RPP = CONT_R // P        # 16

# field ranges per core (start, end); overlaps padded, canonical owner below
FIELD_RANGES = [(0, 5), (5, 10), (10, 15), (15, 20),
                (20, 25), (25, 30), (29, 34), (33, 38)]
# which of the 5 slots are real (not overlap padding)
REAL_SLOTS = [list(range(5))] * 6 + [[1, 2, 3, 4], [1, 2, 3, 4]]

TRACE = False
TRACE_DIR = None     # optional dir for NEFF/NTFF artifacts when TRACE
LAST_RESULT = None   # BassKernelResults of the last run (for profiling)

_NC = None


# packed small-input column layout (int32; f32 sections are bitcast views)
META_IDX = 0
META_XV = SLOTS
META_XC = 2 * SLOTS
META_XV0 = 2 * SLOTS + CONT_RPP
META_WB = 2 * SLOTS + 2 * CONT_RPP
META_COLS = 2 * SLOTS + 2 * CONT_RPP + 2 * EE


def _build_program():
    nc = bacc.Bacc("TRN2", target_bir_lowering=False)

    table = nc.dram_tensor("table", [NF * VOCAB, EE], F32, kind="ExternalInput")
    meta = nc.dram_tensor("meta", [P, META_COLS], I32, kind="ExternalInput")
    out = nc.dram_tensor("out", [P, SLOTS * EE], F32, kind="ExternalOutput")
    outc = nc.dram_tensor("outc", [P, CONT_RPP * EE], F32, kind="ExternalOutput")

    with TileContext(nc) as tc:
        with tc.tile_pool(name="gp", bufs=1) as gp, \
             tc.tile_pool(name="small", bufs=1) as sp:
            # SWDGE load on Pool so the first desc-gen (same engine) follows
            # with no cross-engine sem hop
            meta_t = sp.tile([P, META_COLS], I32)
            nc.gpsimd.dma_start(out=meta_t[:], in_=meta[:])
            idx_ap = meta_t[:, META_IDX:META_IDX + SLOTS]
            xv_ap = meta_t[:, META_XV:META_XV + SLOTS].bitcast(F32)
            xc_ap = meta_t[:, META_XC:META_XC + CONT_RPP].bitcast(F32)
            xv0_ap = meta_t[:, META_XV0:META_XV0 + CONT_RPP].bitcast(F32)
            wb_ap = meta_t[:, META_WB:META_WB + 2 * EE].bitcast(F32)

            # continuous field: outc[p, r, 0:32] = xc*xv0*[w1|w2] + xv0*[b1|b2]
            u_t = sp.tile([P, CONT_RPP], F32)
            nc.vector.tensor_mul(u_t[:], xc_ap, xv0_ap)

            co = sp.tile([P, CONT_RPP * EE], F32)
            co3 = co[:].rearrange("p (r e) -> p r e", e=EE)
            t1 = sp.tile([P, CONT_RPP * EE], F32)
            t13 = t1[:].rearrange("p (r e) -> p r e", e=EE)
            ub = u_t[:].unsqueeze(2).to_broadcast([P, CONT_RPP, EE])
            sb = xv0_ap.unsqueeze(2).to_broadcast([P, CONT_RPP, EE])
            wcb = wb_ap[:, 0:EE].unsqueeze(1).to_broadcast([P, CONT_RPP, EE])
            bcb = wb_ap[:, EE:2 * EE].unsqueeze(1).to_broadcast([P, CONT_RPP, EE])
            nc.vector.tensor_mul(t13, ub, wcb)
            nc.vector.tensor_mul(co3, sb, bcb)
            nc.vector.tensor_add(co3, co3, t13)
            nc.sync.dma_start(out=outc[:], in_=co[:])

            # all gather chunks get distinct tiles; everything fits in SBUF,
            # so the gather stream never stalls on buffer reuse
            off = 0
            for c, jc in enumerate(CHUNKS):
                g = gp.tile([P, jc * EE], F32, tag=f"g{c}")
                g3 = g[:].rearrange("p (j e) -> p j e", e=EE)
                xvb = xv_ap[:, off:off + jc].unsqueeze(2).to_broadcast(
                    [P, jc, EE])
                if FUSE_CCE:
                    # pre-fill tile with broadcast xv; gather multiplies
                    # in-stream via the SDMA CCE unit
                    nc.vector.tensor_copy(g3, xvb)
                nc.gpsimd.indirect_dma_start(
                    out=g[:],
                    out_offset=None,
                    in_=table[:],
                    in_offset=bass.IndirectOffsetOnAxis(
                        ap=idx_ap[:, off:off + jc], axis=0),
                    compute_op=(mybir.AluOpType.mult if FUSE_CCE
                                else mybir.AluOpType.bypass),
                )
                if not FUSE_CCE:
                    nc.vector.tensor_mul(g3, g3, xvb)
                nc.sync.dma_start(
                    out=out[:, off * EE:(off + jc) * EE], in_=g[:])
                off += jc

    nc.compile()
    return nc


def _get_nc():
    global _NC
    if _NC is None:
        _NC = _build_program()
    return _NC


def kernel(Xi, Xv, w1_lin, b1_lin, w2_lin, b2_lin, T1, T2):
    global LAST_RESULT
    Xi = np.asarray(Xi)
    Xv = np.asarray(Xv, dtype=np.float32)
    xi = Xi.reshape(BL, C)          # D=1 squeezed
    xv_flat = Xv.reshape(BL, C)

    # interleaved table: (38, 100000, 32) = [T1 row | T2 row]
    TC = np.empty((F_CAT, VOCAB, EE), dtype=np.float32)
    TC[:, :, :E] = np.asarray(T1, dtype=np.float32)
    TC[:, :, E:] = np.asarray(T2, dtype=np.float32)

    in_maps = []
    for c in range(8):
        a, b = FIELD_RANGES[c]
        tbl = TC[a:b].reshape(NF * VOCAB, EE)

        # flat row indices + xv per lookup slot; slot j of partition p is
        # (row p*RPP + j//NF, field a + j%NF)
        fidx = xi[:, 1 + a:1 + b].astype(np.int64)          # (BL, 5)
        flat = (np.arange(NF, dtype=np.int64)[None, :] * VOCAB + fidx)
        flat = flat.astype(np.int32).reshape(P, SLOTS)
        xvm = xv_flat[:, 1 + a:1 + b].reshape(P, SLOTS).copy()
        # deactivate overlap slots
        dead = [k for k in range(NF) if k not in REAL_SLOTS[c]]
        if dead:
            xvm3 = xvm.reshape(P, RPP, NF)
            for k in dead:
                xvm3[:, :, k] = 0.0
        r0 = c * CONT_R
        xc_m = xi[r0:r0 + CONT_R, 0].astype(np.float32).reshape(P, CONT_RPP)
        xv0_m = xv_flat[r0:r0 + CONT_R, 0].reshape(P, CONT_RPP).copy()
        wbv = np.concatenate([
            np.asarray(w1_lin, np.float32).reshape(E),
            np.asarray(w2_lin, np.float32).reshape(E),
            np.asarray(b1_lin, np.float32).reshape(E),
            np.asarray(b2_lin, np.float32).reshape(E),
        ])
        wb_m = np.broadcast_to(wbv, (P, 2 * EE))
        meta = np.concatenate([
            flat,
            xvm.view(np.int32),
            xc_m.view(np.int32),
            xv0_m.view(np.int32),
            wb_m.view(np.int32),
        ], axis=1)
        in_maps.append({
            "table": np.ascontiguousarray(tbl),
            "meta": np.ascontiguousarray(meta),
        })

    nc = _get_nc()
    kw = {}
    if TRACE and TRACE_DIR:
        kw["tmpdir"] = TRACE_DIR
    res = run_bass_kernel_spmd(nc, in_maps, list(range(8)), trace=TRACE, **kw)
    LAST_RESULT = res
    results = res.results

    emb = np.empty((BL, F_CAT, EE), dtype=np.float32)
    cont = np.empty((BL, EE), dtype=np.float32)
    for c in range(8):
        a, b = FIELD_RANGES[c]
        o = results[c]["out"].reshape(BL, NF, EE)
        for k in REAL_SLOTS[c]:
            emb[:, a + k] = o[:, k]
        r0 = c * CONT_R
        cont[r0:r0 + CONT_R] = results[c]["outc"].reshape(CONT_R, EE)

    first = np.concatenate([cont[:, None, :E], emb[:, :, :E]], axis=1)
    second = np.concatenate([cont[:, None, E:], emb[:, :, E:]], axis=1)
    fm_first = first.reshape(B, L, C * E)
    fm_second = second  # (BL, C, E)
    return fm_first, fm_second


# revision 16
# speedup vs baseline: 1.0938x; 1.0938x over previous
"""DeepFM embedding lookup kernel for 8 TRN2 NeuronCores.

Problem shapes (hardcoded, self-contained):
  Xi (512, 32, 39, 1) int32, Xv (512, 32, 39) f32
  w1_lin/w2_lin (16, 1), b1_lin/b2_lin (16,)
  T1/T2 (38, 100000, 16) f32
Returns (fm_first (512, 32, 624) f32, fm_second (16384, 39, 16) f32)

Sharding: the 38 categorical fields are sharded across the 8 cores, 5
field-slots per core (ranges overlap by one field on cores 6/7; overlap
slots are deactivated with xv=0 and ignored at unshard). T1/T2 rows for
the same (field, vocab) are interleaved host-side into one (.., 32) table
so one 128B gather descriptor fetches both orders' embedding rows.
Each core gathers 16384 rows x 5 fields with one flat row index per
lookup via indirect DMA, scales by Xv on DVE, and writes back. The tiny
continuous-field linear is row-sharded (2048 rows/core).
"""

import numpy as np
import sys

for p in ("/root/.axon_site", "/root/.axon_site/_ro/trn_rl_repo", "/root/.axon_site/_ro/pypackages", "/opt/trn_rl_repo"):
    if p not in sys.path:
        sys.path.append(p)

import concourse.bass as bass
import concourse.bacc as bacc
import concourse.mybir as mybir
from concourse.tile import TileContext
from concourse.bass_utils import run_bass_kernel_spmd

F32 = mybir.dt.float32
I32 = mybir.dt.int32

VOCAB = 100000
E = 16
F_CAT = 38
B, L, C = 512, 32, 39
BL = B * L                    # 16384
P = 128                       # SBUF partitions
NF = 5                        # field slots per core
SLOTS = BL * NF // P          # 640 lookup slots per partition
RPP = BL // P                 # 128 rows per partition
EE = 2 * E                    # 32 f32 per gathered element (T1 row | T2 row)
CHUNKS = [32, 64, 96, 128, 128, 96, 64, 32]   # slots per chunk, sum = SLOTS
assert sum(CHUNKS) == SLOTS
FUSE_CCE = False              # HW: "DMACopy does not support mult with Copy mode"
RC = BL // P                  # rows per partition
CONT_R = BL // 8              # 2048 rows of continuous field per core
CONT_RPP = CONT_R // P        # 16

# field ranges per core (start, end); overlaps padded, canonical owner below
FIELD_RANGES = [(0, 5), (5, 10), (10, 15), (15, 20),
                (20, 25), (25, 30), (29, 34), (33, 38)]
# which of the 5 slots are real (not overlap padding)
REAL_SLOTS = [list(range(5))] * 6 + [[1, 2, 3, 4], [1, 2, 3, 4]]

TRACE = False
TRACE_DIR = None     # optional dir for NEFF/NTFF artifacts when TRACE
LAST_RESULT = None   # BassKernelResults of the last run (for profiling)

_NC = None


# packed small-input column layout (int32; f32 sections are bitcast views)
META_IDX = 0
META_XV = SLOTS
META_XC = 2 * SLOTS
META_XV0 = 2 * SLOTS + CONT_RPP
META_WB = 2 * SLOTS + 2 * CONT_RPP
META_COLS = 2 * SLOTS + 2 * CONT_RPP + 2 * EE


def _build_program():
    nc = bacc.Bacc("TRN2", target_bir_lowering=False)

    table = nc.dram_tensor("table", [NF * VOCAB, EE], F32, kind="ExternalInput")
    meta = nc.dram_tensor("meta", [P, META_COLS], I32, kind="ExternalInput")
    out = nc.dram_tensor("out", [P, SLOTS * EE], F32, kind="ExternalOutput")
    outc = nc.dram_tensor("outc", [P, CONT_RPP * EE], F32, kind="ExternalOutput")

    with TileContext(nc) as tc:
        with tc.tile_pool(name="gp", bufs=1) as gp, \
             tc.tile_pool(name="small", bufs=1) as sp:
            meta_t = sp.tile([P, META_COLS], I32)
            nc.sync.dma_start(out=meta_t[:], in_=meta[:])
            idx_ap = meta_t[:, META_IDX:META_IDX + SLOTS]
            xv_ap = meta_t[:, META_XV:META_XV + SLOTS].bitcast(F32)
            xc_ap = meta_t[:, META_XC:META_XC + CONT_RPP].bitcast(F32)
            xv0_ap = meta_t[:, META_XV0:META_XV0 + CONT_RPP].bitcast(F32)
            wb_ap = meta_t[:, META_WB:META_WB + 2 * EE].bitcast(F32)

            # continuous field: outc[p, r, 0:32] = xc*xv0*[w1|w2] + xv0*[b1|b2]
            u_t = sp.tile([P, CONT_RPP], F32)
            nc.vector.tensor_mul(u_t[:], xc_ap, xv0_ap)

            co = sp.tile([P, CONT_RPP * EE], F32)
            co3 = co[:].rearrange("p (r e) -> p r e", e=EE)
            t1 = sp.tile([P, CONT_RPP * EE], F32)
            t13 = t1[:].rearrange("p (r e) -> p r e", e=EE)
            ub = u_t[:].unsqueeze(2).to_broadcast([P, CONT_RPP, EE])
            sb = xv0_ap.unsqueeze(2).to_broadcast([P, CONT_RPP, EE])
            wcb = wb_ap[:, 0:EE].unsqueeze(1).to_broadcast([P, CONT_RPP, EE])
            bcb = wb_ap[:, EE:2 * EE].unsqueeze(1).to_broadcast([P, CONT_RPP, EE])
            nc.vector.tensor_mul(t13, ub, wcb)
            nc.vector.tensor_mul(co3, sb, bcb)
            nc.vector.tensor_add(co3, co3, t13)
            nc.sync.dma_start(out=outc[:], in_=co[:])

            # all gather chunks get distinct tiles; everything fits in SBUF,
            # so the gather stream never stalls on buffer reuse
            off = 0
            for c, jc in enumerate(CHUNKS):
                g = gp.tile([P, jc * EE], F32, tag=f"g{c}")
                g3 = g[:].rearrange("p (j e) -> p j e", e=EE)
                xvb = xv_ap[:, off:off + jc].unsqueeze(2).to_broadcast(
                    [P, jc, EE])
                if FUSE_CCE:
                    # pre-fill tile with broadcast xv; gather multiplies
                    # in-stream via the SDMA CCE unit
                    nc.vector.tensor_copy(g3, xvb)
                nc.gpsimd.indirect_dma_start(
                    out=g[:],
                    out_offset=None,
                    in_=table[:],
                    in_offset=bass.IndirectOffsetOnAxis(
                        ap=idx_ap[:, off:off + jc], axis=0),
                    compute_op=(mybir.AluOpType.mult if FUSE_CCE
                                else mybir.AluOpType.bypass),
                )
                if not FUSE_CCE:
                    nc.vector.tensor_mul(g3, g3, xvb)
                nc.sync.dma_start(
                    out=out[:, off * EE:(off + jc) * EE], in_=g[:])
                off += jc

    nc.compile()
    return nc


def _get_nc():
    global _NC
    if _NC is None:
        _NC = _build_program()
    return _NC


def kernel(Xi, Xv, w1_lin, b1_lin, w2_lin, b2_lin, T1, T2):
    global LAST_RESULT
    Xi = np.asarray(Xi)
    Xv = np.asarray(Xv, dtype=np.float32)
    xi = Xi.reshape(BL, C)          # D=1 squeezed
    xv_flat = Xv.reshape(BL, C)

    # interleaved table: (38, 100000, 32) = [T1 row | T2 row]
    TC = np.empty((F_CAT, VOCAB, EE), dtype=np.float32)
    TC[:, :, :E] = np.asarray(T1, dtype=np.float32)
    TC[:, :, E:] = np.asarray(T2, dtype=np.float32)

    in_maps = []
    for c in range(8):
        a, b = FIELD_RANGES[c]
        tbl = TC[a:b].reshape(NF * VOCAB, EE)

        # flat row indices + xv per lookup slot; slot j of partition p is
        # (row p*RPP + j//NF, field a + j%NF)
        fidx = xi[:, 1 + a:1 + b].astype(np.int64)          # (BL, 5)
        flat = (np.arange(NF, dtype=np.int64)[None, :] * VOCAB + fidx)
        flat = flat.astype(np.int32).reshape(P, SLOTS)
        xvm = xv_flat[:, 1 + a:1 + b].reshape(P, SLOTS).copy()
        # deactivate overlap slots
        dead = [k for k in range(NF) if k not in REAL_SLOTS[c]]
        if dead:
            xvm3 = xvm.reshape(P, RPP, NF)
            for k in dead:
                xvm3[:, :, k] = 0.0
        r0 = c * CONT_R
        xc_m = xi[r0:r0 + CONT_R, 0].astype(np.float32).reshape(P, CONT_RPP)
        xv0_m = xv_flat[r0:r0 + CONT_R, 0].reshape(P, CONT_RPP).copy()
        wbv = np.concatenate([
            np.asarray(w1_lin, np.float32).reshape(E),
            np.asarray(w2_lin, np.float32).reshape(E),
            np.asarray(b1_lin, np.float32).reshape(E),
            np.asarray(b2_lin, np.float32).reshape(E),
        ])
        wb_m = np.broadcast_to(wbv, (P, 2 * EE))
        meta = np.concatenate([
            flat,
            xvm.view(np.int32),
            xc_m.view(np.int32),
            xv0_m.view(np.int32),
            wb_m.view(np.int32),
        ], axis=1)
        in_maps.append({
            "table": np.ascontiguousarray(tbl),
            "meta": np.ascontiguousarray(meta),
        })

    nc = _get_nc()
    kw = {}
    if TRACE and TRACE_DIR:
        kw["tmpdir"] = TRACE_DIR
    res = run_bass_kernel_spmd(nc, in_maps, list(range(8)), trace=TRACE, **kw)
    LAST_RESULT = res
    results = res.results

    emb = np.empty((BL, F_CAT, EE), dtype=np.float32)
    cont = np.empty((BL, EE), dtype=np.float32)
    for c in range(8):
        a, b = FIELD_RANGES[c]
        o = results[c]["out"].reshape(BL, NF, EE)
        for k in REAL_SLOTS[c]:
            emb[:, a + k] = o[:, k]
        r0 = c * CONT_R
        cont[r0:r0 + CONT_R] = results[c]["outc"].reshape(CONT_R, EE)

    first = np.concatenate([cont[:, None, :E], emb[:, :, :E]], axis=1)
    second = np.concatenate([cont[:, None, E:], emb[:, :, E:]], axis=1)
    fm_first = first.reshape(B, L, C * E)
    fm_second = second  # (BL, C, E)
    return fm_first, fm_second
